# revision 14
# baseline (speedup 1.0000x reference)
"""Trainium2 Bass kernel for nn_Attention_29472065585724.

Reference computation (per batch b of 16, C=1024, H=W=32, seq p2=256, nh=8, hd=512):
    qkv = conv1x1(x, w_qkv, b_qkv)            # [B, 3C, H, W]
    q,k,v = reshape(B, 256, 3, 8, 512) ...    # row-major reshape mixing C and HW
    attn  = softmax(q @ k^T * scale) @ v
    out   = conv1x1(attn_reshaped, w_proj, b_proj)

Strategy (v5):
  - Data-parallel: batch 16 -> 8 cores x 2 batches. No collectives; host gathers.
  - Measured on this HW: f32r matmul streams 512 cols in 227 ns flat even with
    per-instruction stationary reloads; bf16 runs slower in-kernel. The big
    GEMMs (qkv, v, proj) keep f32r operands; only the attention-internal
    tensors (qkT, E, v) are bf16 (frees 48KB/partition, small ST/PV cost).
  - Also measured: k-outer accumulation (8 PSUM banks interleaved) costs
    ~75ns/matmul over back-to-back k-inner chains. So only GEMM1's first
    w1-quarter pass is k-outer - it consumes the (w1-quarter, x-tile) DMA
    pairs in arrival order, starting the PE ~12us into the program - and all
    other GEMM phases are k-inner. Later w1 quarters stream into
    double-buffered slots behind the passes that consume them.
  - w2/wp stay SBUF-resident across both batches; batch 1 replays batch 0's
    schedule into the same SBUF slots, all refill ordering enforced by
    tile-reuse dependencies.
  - b1 is host-replicated to [128, 2048] and DMA'd directly - no PE broadcast
    on the critical path. y is stored bf16 and upcast on host.
  - Host-side weight permutation makes every device layout fall out of plain
    GEMMs with zero on-device transposes (same scheme as v1):
      * q,k produced transposed ([d, seq]) via x^T stationary GEMM; softmax
        scale folded into w_q/b_q.
      * v produced in [seq, d]; proj contraction columns permuted so attention
        outputs land contiguously.
  - Softmax without max-subtraction (S bounded ~|6|); denominator via a tiny
    N=8 matmul of exp(S^T) against ones, normalization during PSUM eviction.
"""
import sys

import numpy as np

if "/opt/trn_rl_repo" not in sys.path:
    sys.path.insert(0, "/opt/trn_rl_repo")

import ml_dtypes

import concourse.bass as bass
import concourse.tile as tile
from concourse import bacc, mybir
from concourse import bass_utils

F32 = mybir.dt.float32
F32R = mybir.dt.float32r
BF16 = mybir.dt.bfloat16
AF = mybir.ActivationFunctionType
BF16_NP = ml_dtypes.bfloat16

B_PER_CORE = 2
N_CORES = 8
CIN = 1024
HW = 1024
NH = 8
P2 = 256
HD = 512

_CACHE = {}


def _build_program():
    nc = bacc.Bacc("TRN2", target_bir_lowering=False, debug=False)
    xl_d = nc.dram_tensor("xl", [B_PER_CORE, CIN, 512], F32R,
                          kind="ExternalInput").ap()
    xh_d = nc.dram_tensor("xh", [B_PER_CORE, CIN, 512], F32R,
                          kind="ExternalInput").ap()
    w1_d = nc.dram_tensor("w1q", [4, CIN, 512], F32R, kind="ExternalInput").ap()
    w2_d = nc.dram_tensor("w2t", [CIN, 1024], F32R, kind="ExternalInput").ap()
    wp_d = nc.dram_tensor("wpt", [1024, 1024], F32R, kind="ExternalInput").ap()
    b1_d = nc.dram_tensor("b1r", [128, 2048], F32, kind="ExternalInput").ap()
    b2_d = nc.dram_tensor("b2", [1024], F32, kind="ExternalInput").ap()
    bp_d = nc.dram_tensor("bp", [1024], F32, kind="ExternalInput").ap()
    ones_d = nc.dram_tensor("ones_c", [128, 8], BF16, kind="ExternalInput").ap()
    y_d = nc.dram_tensor("y", [B_PER_CORE, 1024, HW], BF16, kind="ExternalOutput").ap()

    with tile.TileContext(nc) as tc:
        with tile.ExitStack() as top:
            persist = top.enter_context(tc.tile_pool(name="persist", bufs=1))
            y_pool = top.enter_context(tc.tile_pool(name="ypool", bufs=4))
            w1_pool = top.enter_context(tc.tile_pool(name="w1pool", bufs=1))
            w2_pool = top.enter_context(tc.tile_pool(name="w2pool", bufs=1))

            # tiny constants + replicated b1 on the Activation HWDGE queue
            b2_sb = persist.tile([128, 8], F32, name="b2_sb")
            nc.scalar.dma_start(b2_sb[:], b2_d.rearrange("(t p) -> p t", p=128))
            bp_sb = persist.tile([128, 8], F32, name="bp_sb")
            nc.scalar.dma_start(bp_sb[:], bp_d.rearrange("(t p) -> p t", p=128))
            ones_col = persist.tile([128, 8], BF16, name="ones_col")
            nc.scalar.dma_start(ones_col[:], ones_d[:])
            b1_bc = persist.tile([128, 2048], F32, name="b1_bc")
            nc.scalar.dma_start(b1_bc[:], b1_d[:])

            w2_sb = [w2_pool.tile([128, 1024], F32R, name=f"w2sb{k}", tag=f"w2sb{k}")
                     for k in range(8)]
            wp_pool = top.enter_context(tc.tile_pool(name="wppool", bufs=1))
            wp_sb = [wp_pool.tile([128, 1024], F32R, name=f"wpsb{k}", tag=f"wpsb{k}")
                     for k in range(8)]

            for b in range(B_PER_CORE):
                _emit_batch(nc, tc, b, (xl_d, xh_d), w1_d, w2_d, wp_d, y_d,
                            w1_pool, w2_sb, wp_sb, b1_bc, b2_sb, bp_sb,
                            ones_col, y_pool)
    nc.compile()
    return nc


def _emit_batch(nc, tc, b, x_d, w1_d, w2_d, wp_d, y_d, w1_pool, w2_sb,
                wp_sb, b1_bc, b2_sb, bp_sb, ones_col, y_pool):
    def load_w1_quarter(n):
        w1q = [w1_pool.tile([128, 512], F32R, name=f"w1q{b}_{n}_{k}",
                            tag=f"qbuf{n % 2}_{k}") for k in range(8)]
        for k in range(8):
            nc.sync.dma_start(w1q[k][:], w1_d[n, 128 * k:128 * k + 128, :])
        return w1q

    with tile.ExitStack() as bs:
        qk_pool = bs.enter_context(tc.tile_pool(name=f"qk{b}", bufs=1))
        v_pool = bs.enter_context(tc.tile_pool(name=f"v{b}", bufs=1))
        qkT = [qk_pool.tile([128, 2048], BF16, name=f"qkT{b}_{m}", tag=f"qkT{m}")
               for m in range(8)]
        v_sb = [v_pool.tile([128, 1024], BF16, name=f"vsb{b}_{m}", tag=f"vsb{m}")
                for m in range(8)]

        # ---------------- QKV GEMMs ----------------
        with tile.ExitStack() as qs:
            x_pool = qs.enter_context(tc.tile_pool(name=f"x{b}", bufs=1))
            psg = qs.enter_context(tc.tile_pool(name=f"psg{b}", bufs=8,
                                                space="PSUM"))
            # (w1-quarter0, x) DMA pairs first - the GEMM1 ramp consumes them
            # in arrival order; later quarters stream behind their passes.
            x_sb = [x_pool.tile([128, HW], F32R, name=f"xsb{b}_{k}",
                                tag=f"xsb{k}") for k in range(8)]
            q0 = [w1_pool.tile([128, 512], F32R, name=f"w1q{b}_0_{k}",
                               tag=f"qbuf0_{k}") for k in range(8)]
            xl_d, xh_d = x_d
            for k in range(8):
                nc.sync.dma_start(q0[k][:], w1_d[0, 128 * k:128 * k + 128, :])
                nc.sync.dma_start(x_sb[k][:, 0:512],
                                  xl_d[b, 128 * k:128 * k + 128, :])
                nc.sync.dma_start(x_sb[k][:, 512:1024],
                                  xh_d[b, 128 * k:128 * k + 128, :])
            w1quads = [q0] + [load_w1_quarter(n) for n in range(1, 4)]
            if b == 0:
                for k in range(8):
                    nc.sync.dma_start(w2_sb[k][:], w2_d[128 * k:128 * k + 128, :])
                for k in range(8):
                    nc.sync.dma_start(wp_sb[k][:], wp_d[128 * k:128 * k + 128, :])

            # GEMM1 (q,k): quarter pass 0 k-outer (consumes DMA pairs in
            # arrival order); passes 1-3 k-inner (back-to-back accumulation
            # is ~75ns/matmul faster than bank-interleaved k-outer)
            pss = [psg.tile([128, 512], F32, name=f"psg1_{b}_0_{m}",
                            tag="psg") for m in range(8)]
            for k in range(8):
                for m in range(8):
                    nc.tensor.matmul(
                        pss[m][:],
                        x_sb[k][:, 128 * m:128 * m + 128],
                        w1quads[0][k][:],
                        start=(k == 0), stop=(k == 7))
            for m in range(8):
                nc.vector.tensor_add(qkT[m][:, 0:512], pss[m][:],
                                     b1_bc[:, 0:512])
            for n in range(1, 4):
                w1q = w1quads[n]
                for m in range(8):
                    ps = psg.tile([128, 512], F32, name=f"psg1_{b}_{n}_{m}",
                                  tag="psg")
                    for k in range(8):
                        nc.tensor.matmul(
                            ps[:],
                            x_sb[k][:, 128 * m:128 * m + 128],
                            w1q[k][:],
                            start=(k == 0), stop=(k == 7))
                    nc.vector.tensor_add(qkT[m][:, 512 * n:512 * n + 512],
                                         ps[:], b1_bc[:, 512 * n:512 * n + 512])

            # GEMM2 (v): k-inner
            for m in range(8):
                for n in range(2):
                    ps = psg.tile([128, 512], F32, name=f"psg2_{b}_{m}_{n}",
                                  tag="psg")
                    for k in range(8):
                        nc.tensor.matmul(
                            ps[:],
                            w2_sb[k][:, 128 * m:128 * m + 128],
                            x_sb[k][:, 512 * n:512 * n + 512],
                            start=(k == 0), stop=(k == 7))
                    nc.scalar.activation(v_sb[m][:, 512 * n:512 * n + 512],
                                         ps[:], AF.Identity, bias=b2_sb[:, m:m + 1])

        # ---------------- attention ----------------
        ao_pool = bs.enter_context(tc.tile_pool(name=f"ao{b}", bufs=1))
        ao_sb = [ao_pool.tile([128, 1024], F32R, name=f"aosb{b}_{m}", tag=f"ao{m}")
                 for m in range(8)]

        att = bs.enter_context(tile.ExitStack())
        e_pool = att.enter_context(tc.tile_pool(name=f"e{b}", bufs=2))
        r_pool = att.enter_context(tc.tile_pool(name=f"r{b}", bufs=4))
        ps_st = att.enter_context(tc.tile_pool(name=f"pst{b}", bufs=4, space="PSUM"))
        ps_pv = att.enter_context(tc.tile_pool(name=f"ppv{b}", bufs=2, space="PSUM"))

        def attn_st(h):
            g, half = h // 2, h % 2
            base = 4 * half
            es = []
            for kt in range(2):
                ps = ps_st.tile([128, 256], F32, name=f"ps_st{b}_{h}_{kt}",
                                tag="ps_st")
                for d in range(4):
                    nc.tensor.matmul(
                        ps[:],
                        qkT[base + d][:, (4 + g) * 256 + 128 * kt:
                                      (4 + g) * 256 + 128 * kt + 128],
                        qkT[base + d][:, g * 256:g * 256 + 256],
                        start=(d == 0), stop=(d == 3))
                e = e_pool.tile([128, 256], BF16, name=f"E{b}_{h}_{kt}",
                                tag=f"E{kt}")
                nc.scalar.activation(e[:], ps[:], AF.Exp)
                es.append(e)
            return es

        def attn_pv(h, es):
            g, half = h // 2, h % 2
            for qt in range(2):
                psO = ps_pv.tile([128, 512], F32, name=f"psO{b}_{h}_{qt}", tag="psO")
                psL = ps_pv.tile([128, 8], F32, name=f"psL{b}_{h}_{qt}", tag="psL")
                # bf16 512-free matmuls run 259ns but 256-free run full speed
                # (113.5ns) on this HW - two half-width chains win 32ns each
                for h2 in range(2):
                    for kt in range(2):
                        nc.tensor.matmul(
                            psO[:, 256 * h2:256 * h2 + 256],
                            es[kt][:, 128 * qt:128 * qt + 128],
                            v_sb[2 * g + kt][:, 512 * half + 256 * h2:
                                             512 * half + 256 * h2 + 256],
                            start=(kt == 0), stop=(kt == 1))
                for kt in range(2):
                    nc.tensor.matmul(
                        psL[:], es[kt][:, 128 * qt:128 * qt + 128],
                        ones_col[:, 0:8],
                        start=(kt == 0), stop=(kt == 1))
                r = r_pool.tile([128, 1], F32, name=f"r{b}_{h}_{qt}", tag="r")
                nc.vector.reciprocal(r[:], psL[:, 0:1])
                dst = ao_sb[2 * g + qt]
                nc.vector.tensor_scalar_mul(
                    dst[:, 512 * half:512 * half + 512], psO[:], r[:])

        es_next = attn_st(0)
        for h in range(NH):
            es_cur = es_next
            es_next = attn_st(h + 1) if h + 1 < NH else None
            attn_pv(h, es_cur)
        att.close()

        # ---------------- proj GEMM: k-inner ----------------
        with tile.ExitStack() as pjs:
            psp = pjs.enter_context(tc.tile_pool(name=f"psp{b}", bufs=4,
                                                 space="PSUM"))
            for m in range(8):
                for n in range(2):
                    ps = psp.tile([128, 512], F32, name=f"psp{b}_{m}_{n}",
                                  tag="psp")
                    for k in range(8):
                        nc.tensor.matmul(
                            ps[:],
                            wp_sb[k][:, 128 * m:128 * m + 128],
                            ao_sb[k][:, 512 * n:512 * n + 512],
                            start=(k == 0), stop=(k == 7))
                    y_sb = y_pool.tile([128, 512], BF16, name=f"ysb{b}_{m}_{n}",
                                       tag="ysb")
                    if (2 * m + n) % 2 == 0:
                        nc.scalar.activation(y_sb[:], ps[:], AF.Identity,
                                             bias=bp_sb[:, m:m + 1])
                    else:
                        nc.vector.tensor_scalar_add(y_sb[:], ps[:],
                                                    bp_sb[:, m:m + 1])
                    nc.sync.dma_start(
                        y_d[b, 128 * m:128 * m + 128, 512 * n:512 * n + 512],
                        y_sb[:])


def _prepare_host_inputs(w_qkv, b_qkv, w_proj):
    """Permute weights so device layouts need no transposes. See layout notes."""
    C = CIN
    scale = np.float32((C // NH) ** -0.5)
    g_i, p_i = np.meshgrid(np.arange(4), np.arange(256), indexing="ij")
    # GEMM1 columns: (t, g, p) -> channel 12p + 4t + g
    t_i, g2_i, p2_i = np.meshgrid(np.arange(2), np.arange(4), np.arange(256),
                                  indexing="ij")
    src1 = (12 * p2_i + 4 * t_i + g2_i).reshape(-1)
    w1 = w_qkv[src1, :].astype(np.float32).copy()
    b1 = b_qkv[src1].astype(np.float32).copy()
    w1[:1024] *= scale
    b1[:1024] *= scale
    w1t = np.ascontiguousarray(w1.T)                       # [1024, 2048]
    # GEMM2 rows: r = g*256 + p -> channel 12p + 8 + g
    src2 = (12 * p_i + 8 + g_i).reshape(-1)
    w2t = np.ascontiguousarray(w_qkv[src2, :].T.astype(np.float32))   # [1024, 1024]
    b2 = b_qkv[src2].astype(np.float32).copy()
    # proj contraction: c' = g*256 + p -> orig col 4p + g
    srcp = (4 * p_i + g_i).reshape(-1)
    wpt = np.ascontiguousarray(w_proj[:, srcp].T.astype(np.float32))  # [1024, 1024]
    return w1t, b1, w2t, b2, wpt


def kernel(x, w_qkv, b_qkv, w_proj, b_proj):
    if "nc" not in _CACHE:
        _CACHE["nc"] = _build_program()
    nc = _CACHE["nc"]

    x = np.asarray(x, dtype=np.float32)
    B = x.shape[0]
    xf = x.reshape(B, CIN, HW)
    xl = np.ascontiguousarray(xf[:, :, 0:512])
    xh = np.ascontiguousarray(xf[:, :, 512:1024])
    w1t, b1, w2t, b2, wpt = _prepare_host_inputs(
        np.asarray(w_qkv, np.float32), np.asarray(b_qkv, np.float32),
        np.asarray(w_proj, np.float32))
    # w1 as four contiguous quarter tensors [4, 1024, 512]
    w1q = np.ascontiguousarray(w1t.reshape(CIN, 4, 512).transpose(1, 0, 2))
    b1r = np.ascontiguousarray(np.tile(b1.reshape(1, 2048), (128, 1)))
    bp = np.asarray(b_proj, np.float32)
    ones_c = np.ones((128, 8), BF16_NP)

    in_maps = []
    for c in range(N_CORES):
        in_maps.append({
            "xl": xl[c * B_PER_CORE:(c + 1) * B_PER_CORE],
            "xh": xh[c * B_PER_CORE:(c + 1) * B_PER_CORE],
            "w1q": w1q, "w2t": w2t, "wpt": wpt,
            "b1r": b1r, "b2": b2, "bp": bp,
            "ones_c": ones_c,
        })
    res = bass_utils.run_bass_kernel_spmd(nc, in_maps, core_ids=list(range(N_CORES)))
    _CACHE["last_results"] = res
    y = np.concatenate([np.asarray(res.results[c]["y"], dtype=np.float32)
                        for c in range(N_CORES)], axis=0)
    return np.ascontiguousarray(y.reshape(B, CIN, 32, 32))


# revision 15
# speedup vs baseline: 1.0106x; 1.0106x over previous
"""Trainium2 Bass kernel for nn_Attention_29472065585724.

Reference computation (per batch b of 16, C=1024, H=W=32, seq p2=256, nh=8, hd=512):
    qkv = conv1x1(x, w_qkv, b_qkv)            # [B, 3C, H, W]
    q,k,v = reshape(B, 256, 3, 8, 512) ...    # row-major reshape mixing C and HW
    attn  = softmax(q @ k^T * scale) @ v
    out   = conv1x1(attn_reshaped, w_proj, b_proj)

Strategy (v5):
  - Data-parallel: batch 16 -> 8 cores x 2 batches. No collectives; host gathers.
  - Measured on this HW: f32r matmul streams 512 cols in 227 ns flat even with
    per-instruction stationary reloads; bf16 runs slower in-kernel. The big
    GEMMs (qkv, v, proj) keep f32r operands; only the attention-internal
    tensors (qkT, E, v) are bf16 (frees 48KB/partition, small ST/PV cost).
  - Also measured: k-outer accumulation (8 PSUM banks interleaved) costs
    ~75ns/matmul over back-to-back k-inner chains. So only GEMM1's first
    w1-quarter pass is k-outer - it consumes the (w1-quarter, x-tile) DMA
    pairs in arrival order, starting the PE ~12us into the program - and all
    other GEMM phases are k-inner. Later w1 quarters stream into
    double-buffered slots behind the passes that consume them.
  - w2/wp stay SBUF-resident across both batches; batch 1 replays batch 0's
    schedule into the same SBUF slots, all refill ordering enforced by
    tile-reuse dependencies.
  - b1 is host-replicated to [128, 2048] and DMA'd directly - no PE broadcast
    on the critical path. y is stored bf16 and upcast on host.
  - Host-side weight permutation makes every device layout fall out of plain
    GEMMs with zero on-device transposes (same scheme as v1):
      * q,k produced transposed ([d, seq]) via x^T stationary GEMM; softmax
        scale folded into w_q/b_q.
      * v produced in [seq, d]; proj contraction columns permuted so attention
        outputs land contiguously.
  - Softmax without max-subtraction (S bounded ~|6|); denominator via a tiny
    N=8 matmul of exp(S^T) against ones, normalization during PSUM eviction.
"""
import sys

import numpy as np

if "/opt/trn_rl_repo" not in sys.path:
    sys.path.insert(0, "/opt/trn_rl_repo")

import ml_dtypes

import concourse.bass as bass
import concourse.tile as tile
from concourse import bacc, mybir
from concourse import bass_utils

F32 = mybir.dt.float32
F32R = mybir.dt.float32r
BF16 = mybir.dt.bfloat16
AF = mybir.ActivationFunctionType
BF16_NP = ml_dtypes.bfloat16

B_PER_CORE = 2
N_CORES = 8
CIN = 1024
HW = 1024
NH = 8
P2 = 256
HD = 512

_CACHE = {}


def _build_program():
    nc = bacc.Bacc("TRN2", target_bir_lowering=False, debug=False)
    xl_d = nc.dram_tensor("xl", [B_PER_CORE, CIN, 512], F32R,
                          kind="ExternalInput").ap()
    xh_d = nc.dram_tensor("xh", [B_PER_CORE, CIN, 512], F32R,
                          kind="ExternalInput").ap()
    w1_d = nc.dram_tensor("w1q", [4, CIN, 512], F32R, kind="ExternalInput").ap()
    w2_d = nc.dram_tensor("w2t", [CIN, 1024], F32R, kind="ExternalInput").ap()
    wp_d = nc.dram_tensor("wpt", [1024, 1024], F32R, kind="ExternalInput").ap()
    b1_d = nc.dram_tensor("b1r", [128, 2048], F32, kind="ExternalInput").ap()
    b2_d = nc.dram_tensor("b2", [1024], F32, kind="ExternalInput").ap()
    bp_d = nc.dram_tensor("bp", [1024], F32, kind="ExternalInput").ap()
    ones_d = nc.dram_tensor("ones_c", [128, 8], BF16, kind="ExternalInput").ap()
    y_d = nc.dram_tensor("y", [B_PER_CORE, 1024, HW], BF16, kind="ExternalOutput").ap()

    with tile.TileContext(nc) as tc:
        with tile.ExitStack() as top:
            persist = top.enter_context(tc.tile_pool(name="persist", bufs=1))
            y_pool = top.enter_context(tc.tile_pool(name="ypool", bufs=4))
            w1_pool = top.enter_context(tc.tile_pool(name="w1pool", bufs=1))
            w2_pool = top.enter_context(tc.tile_pool(name="w2pool", bufs=1))

            # tiny constants + replicated b1 on the Activation HWDGE queue
            b2_sb = persist.tile([128, 8], F32, name="b2_sb")
            nc.scalar.dma_start(b2_sb[:], b2_d.rearrange("(t p) -> p t", p=128))
            bp_sb = persist.tile([128, 8], F32, name="bp_sb")
            nc.scalar.dma_start(bp_sb[:], bp_d.rearrange("(t p) -> p t", p=128))
            ones_col = persist.tile([128, 8], BF16, name="ones_col")
            nc.scalar.dma_start(ones_col[:], ones_d[:])
            b1_bc = persist.tile([128, 2048], F32, name="b1_bc")
            nc.scalar.dma_start(b1_bc[:], b1_d[:])

            w2_sb = [w2_pool.tile([128, 1024], F32R, name=f"w2sb{k}", tag=f"w2sb{k}")
                     for k in range(8)]
            wp_pool = top.enter_context(tc.tile_pool(name="wppool", bufs=1))
            wp_sb = [wp_pool.tile([128, 1024], F32R, name=f"wpsb{k}", tag=f"wpsb{k}")
                     for k in range(8)]

            for b in range(B_PER_CORE):
                _emit_batch(nc, tc, b, (xl_d, xh_d), w1_d, w2_d, wp_d, y_d,
                            w1_pool, w2_sb, wp_sb, b1_bc, b2_sb, bp_sb,
                            ones_col, y_pool)
    nc.compile()
    return nc


def _emit_batch(nc, tc, b, x_d, w1_d, w2_d, wp_d, y_d, w1_pool, w2_sb,
                wp_sb, b1_bc, b2_sb, bp_sb, ones_col, y_pool):
    def load_w1_quarter(n):
        w1q = [w1_pool.tile([128, 512], F32R, name=f"w1q{b}_{n}_{k}",
                            tag=f"qbuf{n % 2}_{k}") for k in range(8)]
        for k in range(8):
            nc.sync.dma_start(w1q[k][:], w1_d[n, 128 * k:128 * k + 128, :])
        return w1q

    with tile.ExitStack() as bs:
        qk_pool = bs.enter_context(tc.tile_pool(name=f"qk{b}", bufs=1))
        v_pool = bs.enter_context(tc.tile_pool(name=f"v{b}", bufs=1))
        qkT = [qk_pool.tile([128, 2048], BF16, name=f"qkT{b}_{m}", tag=f"qkT{m}")
               for m in range(8)]
        v_sb = [v_pool.tile([128, 1024], BF16, name=f"vsb{b}_{m}", tag=f"vsb{m}")
                for m in range(8)]

        # ---------------- QKV GEMMs ----------------
        with tile.ExitStack() as qs:
            x_pool = qs.enter_context(tc.tile_pool(name=f"x{b}", bufs=1))
            psg = qs.enter_context(tc.tile_pool(name=f"psg{b}", bufs=8,
                                                space="PSUM"))
            # (w1-quarter0, x) DMA pairs first - the GEMM1 ramp consumes them
            # in arrival order; later quarters stream behind their passes.
            x_sb = [x_pool.tile([128, HW], F32R, name=f"xsb{b}_{k}",
                                tag=f"xsb{k}") for k in range(8)]
            q0 = [w1_pool.tile([128, 512], F32R, name=f"w1q{b}_0_{k}",
                               tag=f"qbuf0_{k}") for k in range(8)]
            xl_d, xh_d = x_d
            for k in range(8):
                nc.sync.dma_start(q0[k][:], w1_d[0, 128 * k:128 * k + 128, :])
                nc.sync.dma_start(x_sb[k][:, 0:512],
                                  xl_d[b, 128 * k:128 * k + 128, :])
                nc.sync.dma_start(x_sb[k][:, 512:1024],
                                  xh_d[b, 128 * k:128 * k + 128, :])
            w1quads = [q0] + [load_w1_quarter(n) for n in range(1, 4)]
            if b == 0:
                for k in range(8):
                    nc.sync.dma_start(w2_sb[k][:], w2_d[128 * k:128 * k + 128, :])
                for k in range(8):
                    nc.sync.dma_start(wp_sb[k][:], wp_d[128 * k:128 * k + 128, :])

            # GEMM1 (q,k): quarter pass 0 k-outer (consumes DMA pairs in
            # arrival order); passes 1-3 k-inner (back-to-back accumulation
            # is ~75ns/matmul faster than bank-interleaved k-outer)
            pss = [psg.tile([128, 512], F32, name=f"psg1_{b}_0_{m}",
                            tag="psg") for m in range(8)]
            for k in range(8):
                for m in range(8):
                    nc.tensor.matmul(
                        pss[m][:],
                        x_sb[k][:, 128 * m:128 * m + 128],
                        w1quads[0][k][:],
                        start=(k == 0), stop=(k == 7))
            for m in range(8):
                nc.vector.tensor_add(qkT[m][:, 0:512], pss[m][:],
                                     b1_bc[:, 0:512])
            for n in range(1, 4):
                w1q = w1quads[n]
                for m in range(8):
                    ps = psg.tile([128, 512], F32, name=f"psg1_{b}_{n}_{m}",
                                  tag="psg")
                    for k in range(8):
                        nc.tensor.matmul(
                            ps[:],
                            x_sb[k][:, 128 * m:128 * m + 128],
                            w1q[k][:],
                            start=(k == 0), stop=(k == 7))
                    nc.vector.tensor_add(qkT[m][:, 512 * n:512 * n + 512],
                                         ps[:], b1_bc[:, 512 * n:512 * n + 512])

            # GEMM2 (v): k-inner
            for m in range(8):
                for n in range(2):
                    ps = psg.tile([128, 512], F32, name=f"psg2_{b}_{m}_{n}",
                                  tag="psg")
                    for k in range(8):
                        nc.tensor.matmul(
                            ps[:],
                            w2_sb[k][:, 128 * m:128 * m + 128],
                            x_sb[k][:, 512 * n:512 * n + 512],
                            start=(k == 0), stop=(k == 7))
                    nc.scalar.activation(v_sb[m][:, 512 * n:512 * n + 512],
                                         ps[:], AF.Identity, bias=b2_sb[:, m:m + 1])

        # ---------------- attention ----------------
        ao_pool = bs.enter_context(tc.tile_pool(name=f"ao{b}", bufs=1))
        ao_sb = [ao_pool.tile([128, 1024], F32R, name=f"aosb{b}_{m}", tag=f"ao{m}")
                 for m in range(8)]

        att = bs.enter_context(tile.ExitStack())
        e_pool = att.enter_context(tc.tile_pool(name=f"e{b}", bufs=2))
        r_pool = att.enter_context(tc.tile_pool(name=f"r{b}", bufs=4))
        ps_st = att.enter_context(tc.tile_pool(name=f"pst{b}", bufs=4, space="PSUM"))
        ps_pv = att.enter_context(tc.tile_pool(name=f"ppv{b}", bufs=2, space="PSUM"))

        def attn_st(h):
            g, half = h // 2, h % 2
            base = 4 * half
            es = []
            for kt in range(2):
                ps = ps_st.tile([128, 256], F32, name=f"ps_st{b}_{h}_{kt}",
                                tag="ps_st")
                for d in range(4):
                    nc.tensor.matmul(
                        ps[:],
                        qkT[base + d][:, (4 + g) * 256 + 128 * kt:
                                      (4 + g) * 256 + 128 * kt + 128],
                        qkT[base + d][:, g * 256:g * 256 + 256],
                        start=(d == 0), stop=(d == 3))
                e = e_pool.tile([128, 256], BF16, name=f"E{b}_{h}_{kt}",
                                tag=f"E{kt}")
                nc.scalar.activation(e[:], ps[:], AF.Exp)
                es.append(e)
            return es

        def attn_pv(h, es):
            g, half = h // 2, h % 2
            for qt in range(2):
                psO = ps_pv.tile([128, 512], F32, name=f"psO{b}_{h}_{qt}", tag="psO")
                psL = ps_pv.tile([128, 8], F32, name=f"psL{b}_{h}_{qt}", tag="psL")
                for kt in range(2):
                    nc.tensor.matmul(
                        psO[:], es[kt][:, 128 * qt:128 * qt + 128],
                        v_sb[2 * g + kt][:, 512 * half:512 * half + 512],
                        start=(kt == 0), stop=(kt == 1))
                    nc.tensor.matmul(
                        psL[:], es[kt][:, 128 * qt:128 * qt + 128],
                        ones_col[:, 0:8],
                        start=(kt == 0), stop=(kt == 1))
                r = r_pool.tile([128, 1], F32, name=f"r{b}_{h}_{qt}", tag="r")
                nc.vector.reciprocal(r[:], psL[:, 0:1])
                dst = ao_sb[2 * g + qt]
                nc.vector.tensor_scalar_mul(
                    dst[:, 512 * half:512 * half + 512], psO[:], r[:])

        es_next = attn_st(0)
        for h in range(NH):
            es_cur = es_next
            es_next = attn_st(h + 1) if h + 1 < NH else None
            attn_pv(h, es_cur)
        att.close()

        # ---------------- proj GEMM: k-inner ----------------
        with tile.ExitStack() as pjs:
            psp = pjs.enter_context(tc.tile_pool(name=f"psp{b}", bufs=4,
                                                 space="PSUM"))
            for m in range(8):
                for n in range(2):
                    ps = psp.tile([128, 512], F32, name=f"psp{b}_{m}_{n}",
                                  tag="psp")
                    for k in range(8):
                        nc.tensor.matmul(
                            ps[:],
                            wp_sb[k][:, 128 * m:128 * m + 128],
                            ao_sb[k][:, 512 * n:512 * n + 512],
                            start=(k == 0), stop=(k == 7))
                    y_sb = y_pool.tile([128, 512], BF16, name=f"ysb{b}_{m}_{n}",
                                       tag="ysb")
                    if (2 * m + n) % 2 == 0:
                        nc.scalar.activation(y_sb[:], ps[:], AF.Identity,
                                             bias=bp_sb[:, m:m + 1])
                    else:
                        nc.vector.tensor_scalar_add(y_sb[:], ps[:],
                                                    bp_sb[:, m:m + 1])
                    nc.sync.dma_start(
                        y_d[b, 128 * m:128 * m + 128, 512 * n:512 * n + 512],
                        y_sb[:])


def _prepare_host_inputs(w_qkv, b_qkv, w_proj):
    """Permute weights so device layouts need no transposes. See layout notes."""
    C = CIN
    scale = np.float32((C // NH) ** -0.5)
    g_i, p_i = np.meshgrid(np.arange(4), np.arange(256), indexing="ij")
    # GEMM1 columns: (t, g, p) -> channel 12p + 4t + g
    t_i, g2_i, p2_i = np.meshgrid(np.arange(2), np.arange(4), np.arange(256),
                                  indexing="ij")
    src1 = (12 * p2_i + 4 * t_i + g2_i).reshape(-1)
    w1 = w_qkv[src1, :].astype(np.float32).copy()
    b1 = b_qkv[src1].astype(np.float32).copy()
    w1[:1024] *= scale
    b1[:1024] *= scale
    w1t = np.ascontiguousarray(w1.T)                       # [1024, 2048]
    # GEMM2 rows: r = g*256 + p -> channel 12p + 8 + g
    src2 = (12 * p_i + 8 + g_i).reshape(-1)
    w2t = np.ascontiguousarray(w_qkv[src2, :].T.astype(np.float32))   # [1024, 1024]
    b2 = b_qkv[src2].astype(np.float32).copy()
    # proj contraction: c' = g*256 + p -> orig col 4p + g
    srcp = (4 * p_i + g_i).reshape(-1)
    wpt = np.ascontiguousarray(w_proj[:, srcp].T.astype(np.float32))  # [1024, 1024]
    return w1t, b1, w2t, b2, wpt


def kernel(x, w_qkv, b_qkv, w_proj, b_proj):
    if "nc" not in _CACHE:
        _CACHE["nc"] = _build_program()
    nc = _CACHE["nc"]

    x = np.asarray(x, dtype=np.float32)
    B = x.shape[0]
    xf = x.reshape(B, CIN, HW)
    xl = np.ascontiguousarray(xf[:, :, 0:512])
    xh = np.ascontiguousarray(xf[:, :, 512:1024])
    w1t, b1, w2t, b2, wpt = _prepare_host_inputs(
        np.asarray(w_qkv, np.float32), np.asarray(b_qkv, np.float32),
        np.asarray(w_proj, np.float32))
    # w1 as four contiguous quarter tensors [4, 1024, 512]
    w1q = np.ascontiguousarray(w1t.reshape(CIN, 4, 512).transpose(1, 0, 2))
    b1r = np.ascontiguousarray(np.tile(b1.reshape(1, 2048), (128, 1)))
    bp = np.asarray(b_proj, np.float32)
    ones_c = np.ones((128, 8), BF16_NP)

    in_maps = []
    for c in range(N_CORES):
        in_maps.append({
            "xl": xl[c * B_PER_CORE:(c + 1) * B_PER_CORE],
            "xh": xh[c * B_PER_CORE:(c + 1) * B_PER_CORE],
            "w1q": w1q, "w2t": w2t, "wpt": wpt,
            "b1r": b1r, "b2": b2, "bp": bp,
            "ones_c": ones_c,
        })
    res = bass_utils.run_bass_kernel_spmd(nc, in_maps, core_ids=list(range(N_CORES)))
    _CACHE["last_results"] = res
    y = np.concatenate([np.asarray(res.results[c]["y"], dtype=np.float32)
                        for c in range(N_CORES)], axis=0)
    return np.ascontiguousarray(y.reshape(B, CIN, 32, 32))


# revision 16
# speedup vs baseline: 1.0228x; 1.0120x over previous
"""Trainium2 Bass kernel for nn_Attention_29472065585724.

Reference computation (per batch b of 16, C=1024, H=W=32, seq p2=256, nh=8, hd=512):
    qkv = conv1x1(x, w_qkv, b_qkv)            # [B, 3C, H, W]
    q,k,v = reshape(B, 256, 3, 8, 512) ...    # row-major reshape mixing C and HW
    attn  = softmax(q @ k^T * scale) @ v
    out   = conv1x1(attn_reshaped, w_proj, b_proj)

Strategy (v5):
  - Data-parallel: batch 16 -> 8 cores x 2 batches. No collectives; host gathers.
  - Measured on this HW: f32r matmul streams 512 cols in 227 ns flat even with
    per-instruction stationary reloads; bf16 runs slower in-kernel. The big
    GEMMs (qkv, v, proj) keep f32r operands; only the attention-internal
    tensors (qkT, E, v) are bf16 (frees 48KB/partition, small ST/PV cost).
  - Also measured: k-outer accumulation (8 PSUM banks interleaved) costs
    ~75ns/matmul over back-to-back k-inner chains. So only GEMM1's first
    w1-quarter pass is k-outer - it consumes the (w1-quarter, x-tile) DMA
    pairs in arrival order, starting the PE ~12us into the program - and all
    other GEMM phases are k-inner. Later w1 quarters stream into
    double-buffered slots behind the passes that consume them.
  - w2/wp stay SBUF-resident across both batches; batch 1 replays batch 0's
    schedule into the same SBUF slots, all refill ordering enforced by
    tile-reuse dependencies.
  - b1 is host-replicated to [128, 2048] and DMA'd directly - no PE broadcast
    on the critical path. y is stored bf16 and upcast on host.
  - Host-side weight permutation makes every device layout fall out of plain
    GEMMs with zero on-device transposes (same scheme as v1):
      * q,k produced transposed ([d, seq]) via x^T stationary GEMM; softmax
        scale folded into w_q/b_q.
      * v produced in [seq, d]; proj contraction columns permuted so attention
        outputs land contiguously.
  - Softmax without max-subtraction (S bounded ~|6|); denominator via a tiny
    N=8 matmul of exp(S^T) against ones, normalization during PSUM eviction.
"""
import sys

import numpy as np

if "/opt/trn_rl_repo" not in sys.path:
    sys.path.insert(0, "/opt/trn_rl_repo")

import ml_dtypes

import concourse.bass as bass
import concourse.tile as tile
from concourse import bacc, mybir
from concourse import bass_utils

F32 = mybir.dt.float32
F32R = mybir.dt.float32r
BF16 = mybir.dt.bfloat16
AF = mybir.ActivationFunctionType
BF16_NP = ml_dtypes.bfloat16

B_PER_CORE = 2
N_CORES = 8
CIN = 1024
HW = 1024
NH = 8
P2 = 256
HD = 512

_CACHE = {}


def _build_program():
    nc = bacc.Bacc("TRN2", target_bir_lowering=False, debug=False)
    xl_d = nc.dram_tensor("xl", [B_PER_CORE, CIN, 512], F32R,
                          kind="ExternalInput").ap()
    xh_d = nc.dram_tensor("xh", [B_PER_CORE, CIN, 512], F32R,
                          kind="ExternalInput").ap()
    w1_d = nc.dram_tensor("w1q", [4, CIN, 512], F32R, kind="ExternalInput").ap()
    w2_d = nc.dram_tensor("w2t", [CIN, 1024], F32R, kind="ExternalInput").ap()
    wp_d = nc.dram_tensor("wpt", [1024, 1024], F32R, kind="ExternalInput").ap()
    b1_d = nc.dram_tensor("b1r", [128, 2048], F32, kind="ExternalInput").ap()
    b2_d = nc.dram_tensor("b2", [1024], F32, kind="ExternalInput").ap()
    bp_d = nc.dram_tensor("bp", [1024], F32, kind="ExternalInput").ap()
    ones_d = nc.dram_tensor("ones_c", [128, 8], F32R, kind="ExternalInput").ap()
    y_d = nc.dram_tensor("y", [B_PER_CORE, 1024, HW], BF16, kind="ExternalOutput").ap()

    with tile.TileContext(nc) as tc:
        with tile.ExitStack() as top:
            persist = top.enter_context(tc.tile_pool(name="persist", bufs=1))
            y_pool = top.enter_context(tc.tile_pool(name="ypool", bufs=2))
            w1_pool = top.enter_context(tc.tile_pool(name="w1pool", bufs=1))
            w2_pool = top.enter_context(tc.tile_pool(name="w2pool", bufs=1))

            # tiny constants + replicated b1 on the Activation HWDGE queue
            b2_sb = persist.tile([128, 8], F32, name="b2_sb")
            nc.scalar.dma_start(b2_sb[:], b2_d.rearrange("(t p) -> p t", p=128))
            bp_sb = persist.tile([128, 8], F32, name="bp_sb")
            nc.scalar.dma_start(bp_sb[:], bp_d.rearrange("(t p) -> p t", p=128))
            ones_col = persist.tile([128, 8], F32R, name="ones_col")
            nc.scalar.dma_start(ones_col[:], ones_d[:])
            b1_bc = persist.tile([128, 2048], F32, name="b1_bc")
            nc.scalar.dma_start(b1_bc[:], b1_d[:])

            w2_sb = [w2_pool.tile([128, 1024], F32R, name=f"w2sb{k}", tag=f"w2sb{k}")
                     for k in range(8)]
            wp_pool = top.enter_context(tc.tile_pool(name="wppool", bufs=1))
            wp_sb = [wp_pool.tile([128, 1024], F32R, name=f"wpsb{k}", tag=f"wpsb{k}")
                     for k in range(8)]

            for b in range(B_PER_CORE):
                _emit_batch(nc, tc, b, (xl_d, xh_d), w1_d, w2_d, wp_d, y_d,
                            w1_pool, w2_sb, wp_sb, b1_bc, b2_sb, bp_sb,
                            ones_col, y_pool)
    nc.compile()
    return nc


def _emit_batch(nc, tc, b, x_d, w1_d, w2_d, wp_d, y_d, w1_pool, w2_sb,
                wp_sb, b1_bc, b2_sb, bp_sb, ones_col, y_pool):
    def load_w1_quarter(n):
        w1q = [w1_pool.tile([128, 512], F32R, name=f"w1q{b}_{n}_{k}",
                            tag=f"qbuf{n % 2}_{k}") for k in range(8)]
        for k in range(8):
            nc.sync.dma_start(w1q[k][:], w1_d[n, 128 * k:128 * k + 128, :])
        return w1q

    with tile.ExitStack() as bs:
        qk_pool = bs.enter_context(tc.tile_pool(name=f"qk{b}", bufs=1))
        v_pool = bs.enter_context(tc.tile_pool(name=f"v{b}", bufs=1))
        qkT = [qk_pool.tile([128, 2048], BF16, name=f"qkT{b}_{m}", tag=f"qkT{m}")
               for m in range(8)]
        v_sb = [v_pool.tile([128, 1024], F32R, name=f"vsb{b}_{m}", tag=f"vsb{m}")
                for m in range(8)]

        # ---------------- QKV GEMMs ----------------
        with tile.ExitStack() as qs:
            x_pool = qs.enter_context(tc.tile_pool(name=f"x{b}", bufs=1))
            psg = qs.enter_context(tc.tile_pool(name=f"psg{b}", bufs=8,
                                                space="PSUM"))
            # (w1-quarter0, x) DMA pairs first - the GEMM1 ramp consumes them
            # in arrival order; later quarters stream behind their passes.
            x_sb = [x_pool.tile([128, HW], F32R, name=f"xsb{b}_{k}",
                                tag=f"xsb{k}") for k in range(8)]
            q0 = [w1_pool.tile([128, 512], F32R, name=f"w1q{b}_0_{k}",
                               tag=f"qbuf0_{k}") for k in range(8)]
            xl_d, xh_d = x_d
            for k in range(8):
                nc.sync.dma_start(q0[k][:], w1_d[0, 128 * k:128 * k + 128, :])
                nc.sync.dma_start(x_sb[k][:, 0:512],
                                  xl_d[b, 128 * k:128 * k + 128, :])
                nc.sync.dma_start(x_sb[k][:, 512:1024],
                                  xh_d[b, 128 * k:128 * k + 128, :])
            w1quads = [q0] + [load_w1_quarter(n) for n in range(1, 4)]
            if b == 0:
                for k in range(8):
                    nc.sync.dma_start(w2_sb[k][:], w2_d[128 * k:128 * k + 128, :])
                for k in range(8):
                    nc.sync.dma_start(wp_sb[k][:], wp_d[128 * k:128 * k + 128, :])

            # GEMM1 (q,k): quarter pass 0 k-outer (consumes DMA pairs in
            # arrival order); passes 1-3 k-inner (back-to-back accumulation
            # is ~75ns/matmul faster than bank-interleaved k-outer)
            pss = [psg.tile([128, 512], F32, name=f"psg1_{b}_0_{m}",
                            tag="psg") for m in range(8)]
            for k in range(8):
                for m in range(8):
                    nc.tensor.matmul(
                        pss[m][:],
                        x_sb[k][:, 128 * m:128 * m + 128],
                        w1quads[0][k][:],
                        start=(k == 0), stop=(k == 7))
            for m in range(8):
                nc.vector.tensor_add(qkT[m][:, 0:512], pss[m][:],
                                     b1_bc[:, 0:512])
            for n in range(1, 4):
                w1q = w1quads[n]
                for m in range(8):
                    ps = psg.tile([128, 512], F32, name=f"psg1_{b}_{n}_{m}",
                                  tag="psg")
                    for k in range(8):
                        nc.tensor.matmul(
                            ps[:],
                            x_sb[k][:, 128 * m:128 * m + 128],
                            w1q[k][:],
                            start=(k == 0), stop=(k == 7))
                    nc.vector.tensor_add(qkT[m][:, 512 * n:512 * n + 512],
                                         ps[:], b1_bc[:, 512 * n:512 * n + 512])

            # GEMM2 (v): k-inner
            for m in range(8):
                for n in range(2):
                    ps = psg.tile([128, 512], F32, name=f"psg2_{b}_{m}_{n}",
                                  tag="psg")
                    for k in range(8):
                        nc.tensor.matmul(
                            ps[:],
                            w2_sb[k][:, 128 * m:128 * m + 128],
                            x_sb[k][:, 512 * n:512 * n + 512],
                            start=(k == 0), stop=(k == 7))
                    nc.scalar.activation(v_sb[m][:, 512 * n:512 * n + 512],
                                         ps[:], AF.Identity, bias=b2_sb[:, m:m + 1])

        # ---------------- attention ----------------
        ao_pool = bs.enter_context(tc.tile_pool(name=f"ao{b}", bufs=1))
        ao_sb = [ao_pool.tile([128, 1024], F32R, name=f"aosb{b}_{m}", tag=f"ao{m}")
                 for m in range(8)]

        att = bs.enter_context(tile.ExitStack())
        e_pool = att.enter_context(tc.tile_pool(name=f"e{b}", bufs=2))
        r_pool = att.enter_context(tc.tile_pool(name=f"r{b}", bufs=4))
        ps_st = att.enter_context(tc.tile_pool(name=f"pst{b}", bufs=4, space="PSUM"))
        ps_pv = att.enter_context(tc.tile_pool(name=f"ppv{b}", bufs=2, space="PSUM"))

        def attn_st(h):
            g, half = h // 2, h % 2
            base = 4 * half
            es = []
            for kt in range(2):
                ps = ps_st.tile([128, 256], F32, name=f"ps_st{b}_{h}_{kt}",
                                tag="ps_st")
                for d in range(4):
                    nc.tensor.matmul(
                        ps[:],
                        qkT[base + d][:, (4 + g) * 256 + 128 * kt:
                                      (4 + g) * 256 + 128 * kt + 128],
                        qkT[base + d][:, g * 256:g * 256 + 256],
                        start=(d == 0), stop=(d == 3))
                e = e_pool.tile([128, 256], F32R, name=f"E{b}_{h}_{kt}",
                                tag=f"E{kt}")
                nc.scalar.activation(e[:], ps[:], AF.Exp)
                es.append(e)
            return es

        def attn_pv(h, es):
            g, half = h // 2, h % 2
            for qt in range(2):
                psO = ps_pv.tile([128, 512], F32, name=f"psO{b}_{h}_{qt}", tag="psO")
                psL = ps_pv.tile([128, 8], F32, name=f"psL{b}_{h}_{qt}", tag="psL")
                for kt in range(2):
                    nc.tensor.matmul(
                        psO[:], es[kt][:, 128 * qt:128 * qt + 128],
                        v_sb[2 * g + kt][:, 512 * half:512 * half + 512],
                        start=(kt == 0), stop=(kt == 1))
                    nc.tensor.matmul(
                        psL[:], es[kt][:, 128 * qt:128 * qt + 128],
                        ones_col[:, 0:8],
                        start=(kt == 0), stop=(kt == 1))
                r = r_pool.tile([128, 1], F32, name=f"r{b}_{h}_{qt}", tag="r")
                nc.vector.reciprocal(r[:], psL[:, 0:1])
                dst = ao_sb[2 * g + qt]
                nc.vector.tensor_scalar_mul(
                    dst[:, 512 * half:512 * half + 512], psO[:], r[:])

        es_next = attn_st(0)
        for h in range(NH):
            es_cur = es_next
            es_next = attn_st(h + 1) if h + 1 < NH else None
            attn_pv(h, es_cur)
        att.close()

        # ---------------- proj GEMM: k-inner ----------------
        with tile.ExitStack() as pjs:
            psp = pjs.enter_context(tc.tile_pool(name=f"psp{b}", bufs=4,
                                                 space="PSUM"))
            for m in range(8):
                for n in range(2):
                    ps = psp.tile([128, 512], F32, name=f"psp{b}_{m}_{n}",
                                  tag="psp")
                    for k in range(8):
                        nc.tensor.matmul(
                            ps[:],
                            wp_sb[k][:, 128 * m:128 * m + 128],
                            ao_sb[k][:, 512 * n:512 * n + 512],
                            start=(k == 0), stop=(k == 7))
                    y_sb = y_pool.tile([128, 512], BF16, name=f"ysb{b}_{m}_{n}",
                                       tag="ysb")
                    if (2 * m + n) % 2 == 0:
                        nc.scalar.activation(y_sb[:], ps[:], AF.Identity,
                                             bias=bp_sb[:, m:m + 1])
                    else:
                        nc.vector.tensor_scalar_add(y_sb[:], ps[:],
                                                    bp_sb[:, m:m + 1])
                    nc.sync.dma_start(
                        y_d[b, 128 * m:128 * m + 128, 512 * n:512 * n + 512],
                        y_sb[:])


def _prepare_host_inputs(w_qkv, b_qkv, w_proj):
    """Permute weights so device layouts need no transposes. See layout notes."""
    C = CIN
    scale = np.float32((C // NH) ** -0.5)
    g_i, p_i = np.meshgrid(np.arange(4), np.arange(256), indexing="ij")
    # GEMM1 columns: (t, g, p) -> channel 12p + 4t + g
    t_i, g2_i, p2_i = np.meshgrid(np.arange(2), np.arange(4), np.arange(256),
                                  indexing="ij")
    src1 = (12 * p2_i + 4 * t_i + g2_i).reshape(-1)
    w1 = w_qkv[src1, :].astype(np.float32).copy()
    b1 = b_qkv[src1].astype(np.float32).copy()
    w1[:1024] *= scale
    b1[:1024] *= scale
    w1t = np.ascontiguousarray(w1.T)                       # [1024, 2048]
    # GEMM2 rows: r = g*256 + p -> channel 12p + 8 + g
    src2 = (12 * p_i + 8 + g_i).reshape(-1)
    w2t = np.ascontiguousarray(w_qkv[src2, :].T.astype(np.float32))   # [1024, 1024]
    b2 = b_qkv[src2].astype(np.float32).copy()
    # proj contraction: c' = g*256 + p -> orig col 4p + g
    srcp = (4 * p_i + g_i).reshape(-1)
    wpt = np.ascontiguousarray(w_proj[:, srcp].T.astype(np.float32))  # [1024, 1024]
    return w1t, b1, w2t, b2, wpt


def kernel(x, w_qkv, b_qkv, w_proj, b_proj):
    if "nc" not in _CACHE:
        _CACHE["nc"] = _build_program()
    nc = _CACHE["nc"]

    x = np.asarray(x, dtype=np.float32)
    B = x.shape[0]
    xf = x.reshape(B, CIN, HW)
    xl = np.ascontiguousarray(xf[:, :, 0:512])
    xh = np.ascontiguousarray(xf[:, :, 512:1024])
    w1t, b1, w2t, b2, wpt = _prepare_host_inputs(
        np.asarray(w_qkv, np.float32), np.asarray(b_qkv, np.float32),
        np.asarray(w_proj, np.float32))
    # w1 as four contiguous quarter tensors [4, 1024, 512]
    w1q = np.ascontiguousarray(w1t.reshape(CIN, 4, 512).transpose(1, 0, 2))
    b1r = np.ascontiguousarray(np.tile(b1.reshape(1, 2048), (128, 1)))
    bp = np.asarray(b_proj, np.float32)
    ones_c = np.ones((128, 8), np.float32)

    in_maps = []
    for c in range(N_CORES):
        in_maps.append({
            "xl": xl[c * B_PER_CORE:(c + 1) * B_PER_CORE],
            "xh": xh[c * B_PER_CORE:(c + 1) * B_PER_CORE],
            "w1q": w1q, "w2t": w2t, "wpt": wpt,
            "b1r": b1r, "b2": b2, "bp": bp,
            "ones_c": ones_c,
        })
    res = bass_utils.run_bass_kernel_spmd(nc, in_maps, core_ids=list(range(N_CORES)))
    _CACHE["last_results"] = res
    y = np.concatenate([np.asarray(res.results[c]["y"], dtype=np.float32)
                        for c in range(N_CORES)], axis=0)
    return np.ascontiguousarray(y.reshape(B, CIN, 32, 32))


# revision 17
# speedup vs baseline: 1.0307x; 1.0078x over previous
"""Trainium2 Bass kernel for nn_Attention_29472065585724.

Reference computation (per batch b of 16, C=1024, H=W=32, seq p2=256, nh=8, hd=512):
    qkv = conv1x1(x, w_qkv, b_qkv)            # [B, 3C, H, W]
    q,k,v = reshape(B, 256, 3, 8, 512) ...    # row-major reshape mixing C and HW
    attn  = softmax(q @ k^T * scale) @ v
    out   = conv1x1(attn_reshaped, w_proj, b_proj)

Strategy (v8):
  - Data-parallel: batch 16 -> 8 cores x 2 batches. No collectives; host gathers.
  - Measured on this HW: f32r matmul streams 512 cols in 227 ns flat even with
    per-instruction stationary reloads; bf16 runs slower in-kernel. The big
    GEMMs (qkv, v, proj) keep f32r operands; only the attention-internal
    tensors (qkT, E, v) are bf16 (frees 48KB/partition, small ST/PV cost).
  - Also measured: k-outer accumulation (8 PSUM banks interleaved) costs
    ~75ns/matmul over back-to-back k-inner chains. So only GEMM1's first
    w1-quarter pass is k-outer - it consumes the (w1-quarter, x-tile) DMA
    pairs in arrival order, starting the PE ~12us into the program - and all
    other GEMM phases are k-inner. Later w1 quarters stream into
    double-buffered slots behind the passes that consume them.
  - w2/wp stay SBUF-resident across both batches; batch 1 replays batch 0's
    schedule into the same SBUF slots, all refill ordering enforced by
    tile-reuse dependencies.
  - b1 is host-replicated to [128, 2048] and DMA'd directly - no PE broadcast
    on the critical path. x is host-split into two contiguous column-half
    tensors so the ramp's first sweep starts as soon as ~500KB lands. y is
    stored bf16 on the idle SP HWDGE queue (fast tail) and upcast on host.
  - Host-side weight permutation makes every device layout fall out of plain
    GEMMs with zero on-device transposes (same scheme as v1):
      * q,k produced transposed ([d, seq]) via x^T stationary GEMM; softmax
        scale folded into w_q/b_q.
      * v produced in [seq, d]; proj contraction columns permuted so attention
        outputs land contiguously.
  - Softmax without max-subtraction (S bounded ~|6|); denominator via a tiny
    N=8 matmul of exp(S^T) against ones, normalization during PSUM eviction.
"""
import sys

import numpy as np

if "/opt/trn_rl_repo" not in sys.path:
    sys.path.insert(0, "/opt/trn_rl_repo")

import ml_dtypes

import concourse.bass as bass
import concourse.tile as tile
from concourse import bacc, mybir
from concourse import bass_utils

F32 = mybir.dt.float32
F32R = mybir.dt.float32r
BF16 = mybir.dt.bfloat16
AF = mybir.ActivationFunctionType
BF16_NP = ml_dtypes.bfloat16

B_PER_CORE = 2
N_CORES = 8
CIN = 1024
HW = 1024
NH = 8
P2 = 256
HD = 512

_CACHE = {}


def _build_program():
    nc = bacc.Bacc("TRN2", target_bir_lowering=False, debug=False)
    xl_d = nc.dram_tensor("xl", [B_PER_CORE, CIN, 512], F32R,
                          kind="ExternalInput").ap()
    xh_d = nc.dram_tensor("xh", [B_PER_CORE, CIN, 512], F32R,
                          kind="ExternalInput").ap()
    w1_d = nc.dram_tensor("w1q", [4, CIN, 512], F32R, kind="ExternalInput").ap()
    w2_d = nc.dram_tensor("w2t", [CIN, 1024], F32R, kind="ExternalInput").ap()
    wp_d = nc.dram_tensor("wpt", [1024, 1024], F32R, kind="ExternalInput").ap()
    b1_d = nc.dram_tensor("b1r", [128, 2048], F32, kind="ExternalInput").ap()
    b2_d = nc.dram_tensor("b2", [1024], F32, kind="ExternalInput").ap()
    bp_d = nc.dram_tensor("bp", [1024], F32, kind="ExternalInput").ap()
    ones_d = nc.dram_tensor("ones_c", [128, 8], BF16, kind="ExternalInput").ap()
    y_d = nc.dram_tensor("y", [B_PER_CORE, 1024, HW], BF16, kind="ExternalOutput").ap()

    with tile.TileContext(nc) as tc:
        with tile.ExitStack() as top:
            persist = top.enter_context(tc.tile_pool(name="persist", bufs=1))
            y_pool = top.enter_context(tc.tile_pool(name="ypool", bufs=4))
            w1_pool = top.enter_context(tc.tile_pool(name="w1pool", bufs=1))
            w2_pool = top.enter_context(tc.tile_pool(name="w2pool", bufs=1))

            # tiny constants + replicated b1 on the Activation HWDGE queue
            b2_sb = persist.tile([128, 8], F32, name="b2_sb")
            nc.scalar.dma_start(b2_sb[:], b2_d.rearrange("(t p) -> p t", p=128))
            bp_sb = persist.tile([128, 8], F32, name="bp_sb")
            nc.scalar.dma_start(bp_sb[:], bp_d.rearrange("(t p) -> p t", p=128))
            ones_col = persist.tile([128, 8], BF16, name="ones_col")
            nc.scalar.dma_start(ones_col[:], ones_d[:])
            b1_bc = persist.tile([128, 2048], F32, name="b1_bc")
            nc.scalar.dma_start(b1_bc[:], b1_d[:])

            w2_sb = [w2_pool.tile([128, 1024], F32R, name=f"w2sb{k}", tag=f"w2sb{k}")
                     for k in range(8)]
            wp_pool = top.enter_context(tc.tile_pool(name="wppool", bufs=1))
            wp_sb = [wp_pool.tile([128, 1024], F32R, name=f"wpsb{k}", tag=f"wpsb{k}")
                     for k in range(8)]

            for b in range(B_PER_CORE):
                _emit_batch(nc, tc, b, (xl_d, xh_d), w1_d, w2_d, wp_d, y_d,
                            w1_pool, w2_sb, wp_sb, b1_bc, b2_sb, bp_sb,
                            ones_col, y_pool)
    nc.compile()
    return nc


def _emit_batch(nc, tc, b, x_d, w1_d, w2_d, wp_d, y_d, w1_pool, w2_sb,
                wp_sb, b1_bc, b2_sb, bp_sb, ones_col, y_pool):
    def load_w1_quarter(n):
        w1q = [w1_pool.tile([128, 512], F32R, name=f"w1q{b}_{n}_{k}",
                            tag=f"qbuf{n % 2}_{k}") for k in range(8)]
        for k in range(8):
            nc.sync.dma_start(w1q[k][:], w1_d[n, 128 * k:128 * k + 128, :])
        return w1q

    with tile.ExitStack() as bs:
        qk_pool = bs.enter_context(tc.tile_pool(name=f"qk{b}", bufs=1))
        v_pool = bs.enter_context(tc.tile_pool(name=f"v{b}", bufs=1))
        qkT = [qk_pool.tile([128, 2048], BF16, name=f"qkT{b}_{m}", tag=f"qkT{m}")
               for m in range(8)]
        v_sb = [v_pool.tile([128, 1024], BF16, name=f"vsb{b}_{m}", tag=f"vsb{m}")
                for m in range(8)]

        # ---------------- QKV GEMMs ----------------
        with tile.ExitStack() as qs:
            x_pool = qs.enter_context(tc.tile_pool(name=f"x{b}", bufs=1))
            psg = qs.enter_context(tc.tile_pool(name=f"psg{b}", bufs=8,
                                                space="PSUM"))
            # (w1-quarter0, x) DMA pairs first - the GEMM1 ramp consumes them
            # in arrival order; later quarters stream behind their passes.
            x_sb = [x_pool.tile([128, HW], F32R, name=f"xsb{b}_{k}",
                                tag=f"xsb{k}") for k in range(8)]
            q0 = [w1_pool.tile([128, 512], F32R, name=f"w1q{b}_0_{k}",
                               tag=f"qbuf0_{k}") for k in range(8)]
            xl_d, xh_d = x_d
            for k in range(8):
                nc.sync.dma_start(q0[k][:], w1_d[0, 128 * k:128 * k + 128, :])
                nc.sync.dma_start(x_sb[k][:, 0:512],
                                  xl_d[b, 128 * k:128 * k + 128, :])
                nc.sync.dma_start(x_sb[k][:, 512:1024],
                                  xh_d[b, 128 * k:128 * k + 128, :])
            w1quads = [q0] + [load_w1_quarter(n) for n in range(1, 4)]
            if b == 0:
                for k in range(8):
                    nc.sync.dma_start(w2_sb[k][:], w2_d[128 * k:128 * k + 128, :])
                for k in range(8):
                    nc.sync.dma_start(wp_sb[k][:], wp_d[128 * k:128 * k + 128, :])

            # GEMM1 (q,k): quarter pass 0 k-outer (consumes DMA pairs in
            # arrival order); passes 1-3 k-inner (back-to-back accumulation
            # is ~75ns/matmul faster than bank-interleaved k-outer)
            pss = [psg.tile([128, 512], F32, name=f"psg1_{b}_0_{m}",
                            tag="psg") for m in range(8)]
            for k in range(8):
                for m in range(8):
                    nc.tensor.matmul(
                        pss[m][:],
                        x_sb[k][:, 128 * m:128 * m + 128],
                        w1quads[0][k][:],
                        start=(k == 0), stop=(k == 7))
            for m in range(8):
                nc.vector.tensor_add(qkT[m][:, 0:512], pss[m][:],
                                     b1_bc[:, 0:512])
            for n in range(1, 4):
                w1q = w1quads[n]
                for m in range(8):
                    ps = psg.tile([128, 512], F32, name=f"psg1_{b}_{n}_{m}",
                                  tag="psg")
                    for k in range(8):
                        nc.tensor.matmul(
                            ps[:],
                            x_sb[k][:, 128 * m:128 * m + 128],
                            w1q[k][:],
                            start=(k == 0), stop=(k == 7))
                    nc.vector.tensor_add(qkT[m][:, 512 * n:512 * n + 512],
                                         ps[:], b1_bc[:, 512 * n:512 * n + 512])

            # GEMM2 (v): k-inner
            for m in range(8):
                for n in range(2):
                    ps = psg.tile([128, 512], F32, name=f"psg2_{b}_{m}_{n}",
                                  tag="psg")
                    for k in range(8):
                        nc.tensor.matmul(
                            ps[:],
                            w2_sb[k][:, 128 * m:128 * m + 128],
                            x_sb[k][:, 512 * n:512 * n + 512],
                            start=(k == 0), stop=(k == 7))
                    nc.scalar.activation(v_sb[m][:, 512 * n:512 * n + 512],
                                         ps[:], AF.Identity, bias=b2_sb[:, m:m + 1])

        # ---------------- attention ----------------
        ao_pool = bs.enter_context(tc.tile_pool(name=f"ao{b}", bufs=1))
        ao_sb = [ao_pool.tile([128, 1024], F32R, name=f"aosb{b}_{m}", tag=f"ao{m}")
                 for m in range(8)]

        att = bs.enter_context(tile.ExitStack())
        e_pool = att.enter_context(tc.tile_pool(name=f"e{b}", bufs=2))
        r_pool = att.enter_context(tc.tile_pool(name=f"r{b}", bufs=4))
        ps_st = att.enter_context(tc.tile_pool(name=f"pst{b}", bufs=4, space="PSUM"))
        ps_pv = att.enter_context(tc.tile_pool(name=f"ppv{b}", bufs=2, space="PSUM"))

        def attn_st(h):
            g, half = h // 2, h % 2
            base = 4 * half
            es = []
            for kt in range(2):
                ps = ps_st.tile([128, 256], F32, name=f"ps_st{b}_{h}_{kt}",
                                tag="ps_st")
                for d in range(4):
                    nc.tensor.matmul(
                        ps[:],
                        qkT[base + d][:, (4 + g) * 256 + 128 * kt:
                                      (4 + g) * 256 + 128 * kt + 128],
                        qkT[base + d][:, g * 256:g * 256 + 256],
                        start=(d == 0), stop=(d == 3))
                e = e_pool.tile([128, 256], BF16, name=f"E{b}_{h}_{kt}",
                                tag=f"E{kt}")
                nc.scalar.activation(e[:], ps[:], AF.Exp)
                es.append(e)
            return es

        def attn_pv(h, es):
            g, half = h // 2, h % 2
            for qt in range(2):
                psO = ps_pv.tile([128, 512], F32, name=f"psO{b}_{h}_{qt}", tag="psO")
                psL = ps_pv.tile([128, 8], F32, name=f"psL{b}_{h}_{qt}", tag="psL")
                for kt in range(2):
                    nc.tensor.matmul(
                        psO[:], es[kt][:, 128 * qt:128 * qt + 128],
                        v_sb[2 * g + kt][:, 512 * half:512 * half + 512],
                        start=(kt == 0), stop=(kt == 1))
                    nc.tensor.matmul(
                        psL[:], es[kt][:, 128 * qt:128 * qt + 128],
                        ones_col[:, 0:8],
                        start=(kt == 0), stop=(kt == 1))
                r = r_pool.tile([128, 1], F32, name=f"r{b}_{h}_{qt}", tag="r")
                nc.vector.reciprocal(r[:], psL[:, 0:1])
                dst = ao_sb[2 * g + qt]
                nc.vector.tensor_scalar_mul(
                    dst[:, 512 * half:512 * half + 512], psO[:], r[:])

        es_next = attn_st(0)
        for h in range(NH):
            es_cur = es_next
            es_next = attn_st(h + 1) if h + 1 < NH else None
            attn_pv(h, es_cur)
        att.close()

        # ---------------- proj GEMM: k-inner ----------------
        with tile.ExitStack() as pjs:
            psp = pjs.enter_context(tc.tile_pool(name=f"psp{b}", bufs=4,
                                                 space="PSUM"))
            for m in range(8):
                for n in range(2):
                    ps = psp.tile([128, 512], F32, name=f"psp{b}_{m}_{n}",
                                  tag="psp")
                    for k in range(8):
                        nc.tensor.matmul(
                            ps[:],
                            wp_sb[k][:, 128 * m:128 * m + 128],
                            ao_sb[k][:, 512 * n:512 * n + 512],
                            start=(k == 0), stop=(k == 7))
                    y_sb = y_pool.tile([128, 512], BF16, name=f"ysb{b}_{m}_{n}",
                                       tag="ysb")
                    if (2 * m + n) % 2 == 0:
                        nc.scalar.activation(y_sb[:], ps[:], AF.Identity,
                                             bias=bp_sb[:, m:m + 1])
                    else:
                        nc.vector.tensor_scalar_add(y_sb[:], ps[:],
                                                    bp_sb[:, m:m + 1])
                    nc.sync.dma_start(
                        y_d[b, 128 * m:128 * m + 128, 512 * n:512 * n + 512],
                        y_sb[:])


def _prepare_host_inputs(w_qkv, b_qkv, w_proj):
    """Permute weights so device layouts need no transposes. See layout notes."""
    C = CIN
    scale = np.float32((C // NH) ** -0.5)
    g_i, p_i = np.meshgrid(np.arange(4), np.arange(256), indexing="ij")
    # GEMM1 columns: (t, g, p) -> channel 12p + 4t + g
    t_i, g2_i, p2_i = np.meshgrid(np.arange(2), np.arange(4), np.arange(256),
                                  indexing="ij")
    src1 = (12 * p2_i + 4 * t_i + g2_i).reshape(-1)
    w1 = w_qkv[src1, :].astype(np.float32).copy()
    b1 = b_qkv[src1].astype(np.float32).copy()
    w1[:1024] *= scale
    b1[:1024] *= scale
    w1t = np.ascontiguousarray(w1.T)                       # [1024, 2048]
    # GEMM2 rows: r = g*256 + p -> channel 12p + 8 + g
    src2 = (12 * p_i + 8 + g_i).reshape(-1)
    w2t = np.ascontiguousarray(w_qkv[src2, :].T.astype(np.float32))   # [1024, 1024]
    b2 = b_qkv[src2].astype(np.float32).copy()
    # proj contraction: c' = g*256 + p -> orig col 4p + g
    srcp = (4 * p_i + g_i).reshape(-1)
    wpt = np.ascontiguousarray(w_proj[:, srcp].T.astype(np.float32))  # [1024, 1024]
    return w1t, b1, w2t, b2, wpt


def kernel(x, w_qkv, b_qkv, w_proj, b_proj):
    if "nc" not in _CACHE:
        _CACHE["nc"] = _build_program()
    nc = _CACHE["nc"]

    x = np.asarray(x, dtype=np.float32)
    B = x.shape[0]
    xf = x.reshape(B, CIN, HW)
    xl = np.ascontiguousarray(xf[:, :, 0:512])
    xh = np.ascontiguousarray(xf[:, :, 512:1024])
    w1t, b1, w2t, b2, wpt = _prepare_host_inputs(
        np.asarray(w_qkv, np.float32), np.asarray(b_qkv, np.float32),
        np.asarray(w_proj, np.float32))
    # w1 as four contiguous quarter tensors [4, 1024, 512]
    w1q = np.ascontiguousarray(w1t.reshape(CIN, 4, 512).transpose(1, 0, 2))
    b1r = np.ascontiguousarray(np.tile(b1.reshape(1, 2048), (128, 1)))
    bp = np.asarray(b_proj, np.float32)
    ones_c = np.ones((128, 8), BF16_NP)

    in_maps = []
    for c in range(N_CORES):
        in_maps.append({
            "xl": xl[c * B_PER_CORE:(c + 1) * B_PER_CORE],
            "xh": xh[c * B_PER_CORE:(c + 1) * B_PER_CORE],
            "w1q": w1q, "w2t": w2t, "wpt": wpt,
            "b1r": b1r, "b2": b2, "bp": bp,
            "ones_c": ones_c,
        })
    res = bass_utils.run_bass_kernel_spmd(nc, in_maps, core_ids=list(range(N_CORES)))
    _CACHE["last_results"] = res
    y = np.concatenate([np.asarray(res.results[c]["y"], dtype=np.float32)
                        for c in range(N_CORES)], axis=0)
    return np.ascontiguousarray(y.reshape(B, CIN, 32, 32))


# revision 18
# speedup vs baseline: 1.0326x; 1.0018x over previous
"""Trainium2 Bass kernel for nn_Attention_29472065585724.

Reference computation (per batch b of 16, C=1024, H=W=32, seq p2=256, nh=8, hd=512):
    qkv = conv1x1(x, w_qkv, b_qkv)            # [B, 3C, H, W]
    q,k,v = reshape(B, 256, 3, 8, 512) ...    # row-major reshape mixing C and HW
    attn  = softmax(q @ k^T * scale) @ v
    out   = conv1x1(attn_reshaped, w_proj, b_proj)

Strategy (v8):
  - Data-parallel: batch 16 -> 8 cores x 2 batches. No collectives; host gathers.
  - Measured on this HW: f32r matmul streams 512 cols in 227 ns flat even with
    per-instruction stationary reloads; bf16 runs slower in-kernel. The big
    GEMMs (qkv, v, proj) keep f32r operands; only the attention-internal
    tensors (qkT, E, v) are bf16 (frees 48KB/partition, small ST/PV cost).
  - Also measured: k-outer accumulation (8 PSUM banks interleaved) costs
    ~75ns/matmul over back-to-back k-inner chains. So only GEMM1's first
    w1-quarter pass is k-outer - it consumes the (w1-quarter, x-tile) DMA
    pairs in arrival order, starting the PE ~12us into the program - and all
    other GEMM phases are k-inner. Later w1 quarters stream into
    double-buffered slots behind the passes that consume them.
  - w2/wp stay SBUF-resident across both batches; batch 1 replays batch 0's
    schedule into the same SBUF slots, all refill ordering enforced by
    tile-reuse dependencies.
  - b1 is host-replicated to [128, 2048] and DMA'd directly - no PE broadcast
    on the critical path. x is host-split into two contiguous column-half
    tensors so the ramp's first sweep starts as soon as ~500KB lands. y is
    stored bf16 on the idle SP HWDGE queue (fast tail) and upcast on host.
  - Host-side weight permutation makes every device layout fall out of plain
    GEMMs with zero on-device transposes (same scheme as v1):
      * q,k produced transposed ([d, seq]) via x^T stationary GEMM; softmax
        scale folded into w_q/b_q.
      * v produced in [seq, d]; proj contraction columns permuted so attention
        outputs land contiguously.
  - Softmax without max-subtraction (S bounded ~|6|); denominator via a tiny
    N=8 matmul of exp(S^T) against ones, normalization during PSUM eviction.
"""
import sys

import numpy as np

if "/opt/trn_rl_repo" not in sys.path:
    sys.path.insert(0, "/opt/trn_rl_repo")

import ml_dtypes

import concourse.bass as bass
import concourse.tile as tile
from concourse import bacc, mybir
from concourse import bass_utils

F32 = mybir.dt.float32
F32R = mybir.dt.float32r
BF16 = mybir.dt.bfloat16
AF = mybir.ActivationFunctionType
BF16_NP = ml_dtypes.bfloat16

B_PER_CORE = 2
N_CORES = 8
CIN = 1024
HW = 1024
NH = 8
P2 = 256
HD = 512

_CACHE = {}


def _build_program():
    nc = bacc.Bacc("TRN2", target_bir_lowering=False, debug=False)
    xl_d = nc.dram_tensor("xl", [B_PER_CORE, CIN, 512], BF16,
                          kind="ExternalInput").ap()
    xh_d = nc.dram_tensor("xh", [B_PER_CORE, CIN, 512], BF16,
                          kind="ExternalInput").ap()
    w1_d = nc.dram_tensor("w1q", [4, CIN, 512], F32R, kind="ExternalInput").ap()
    w2_d = nc.dram_tensor("w2t", [CIN, 1024], F32R, kind="ExternalInput").ap()
    wp_d = nc.dram_tensor("wpt", [1024, 1024], F32R, kind="ExternalInput").ap()
    b1_d = nc.dram_tensor("b1r", [128, 2048], F32, kind="ExternalInput").ap()
    b2_d = nc.dram_tensor("b2", [1024], F32, kind="ExternalInput").ap()
    bp_d = nc.dram_tensor("bp", [1024], F32, kind="ExternalInput").ap()
    ones_d = nc.dram_tensor("ones_c", [128, 8], BF16, kind="ExternalInput").ap()
    y_d = nc.dram_tensor("y", [B_PER_CORE, 1024, HW], BF16, kind="ExternalOutput").ap()

    with tile.TileContext(nc) as tc:
        with tile.ExitStack() as top:
            persist = top.enter_context(tc.tile_pool(name="persist", bufs=1))
            y_pool = top.enter_context(tc.tile_pool(name="ypool", bufs=4))
            w1_pool = top.enter_context(tc.tile_pool(name="w1pool", bufs=1))
            w2_pool = top.enter_context(tc.tile_pool(name="w2pool", bufs=1))

            # tiny constants + replicated b1 on the Activation HWDGE queue
            b2_sb = persist.tile([128, 8], F32, name="b2_sb")
            nc.scalar.dma_start(b2_sb[:], b2_d.rearrange("(t p) -> p t", p=128))
            bp_sb = persist.tile([128, 8], F32, name="bp_sb")
            nc.scalar.dma_start(bp_sb[:], bp_d.rearrange("(t p) -> p t", p=128))
            ones_col = persist.tile([128, 8], BF16, name="ones_col")
            nc.scalar.dma_start(ones_col[:], ones_d[:])
            b1_bc = persist.tile([128, 2048], F32, name="b1_bc")
            nc.scalar.dma_start(b1_bc[:], b1_d[:])

            w2_sb = [w2_pool.tile([128, 1024], F32R, name=f"w2sb{k}", tag=f"w2sb{k}")
                     for k in range(8)]
            wp_pool = top.enter_context(tc.tile_pool(name="wppool", bufs=1))
            wp_sb = [wp_pool.tile([128, 1024], F32R, name=f"wpsb{k}", tag=f"wpsb{k}")
                     for k in range(8)]

            for b in range(B_PER_CORE):
                _emit_batch(nc, tc, b, (xl_d, xh_d), w1_d, w2_d, wp_d, y_d,
                            w1_pool, w2_sb, wp_sb, b1_bc, b2_sb, bp_sb,
                            ones_col, y_pool)
    nc.compile()
    return nc


def _emit_batch(nc, tc, b, x_d, w1_d, w2_d, wp_d, y_d, w1_pool, w2_sb,
                wp_sb, b1_bc, b2_sb, bp_sb, ones_col, y_pool):
    def load_w1_quarter(n):
        w1q = [w1_pool.tile([128, 512], F32R, name=f"w1q{b}_{n}_{k}",
                            tag=f"qbuf{n % 2}_{k}") for k in range(8)]
        for k in range(8):
            nc.sync.dma_start(w1q[k][:], w1_d[n, 128 * k:128 * k + 128, :])
        return w1q

    with tile.ExitStack() as bs:
        qk_pool = bs.enter_context(tc.tile_pool(name=f"qk{b}", bufs=1))
        v_pool = bs.enter_context(tc.tile_pool(name=f"v{b}", bufs=1))
        qkT = [qk_pool.tile([128, 2048], BF16, name=f"qkT{b}_{m}", tag=f"qkT{m}")
               for m in range(8)]
        v_sb = [v_pool.tile([128, 1024], BF16, name=f"vsb{b}_{m}", tag=f"vsb{m}")
                for m in range(8)]

        # ---------------- QKV GEMMs ----------------
        with tile.ExitStack() as qs:
            x_pool = qs.enter_context(tc.tile_pool(name=f"x{b}", bufs=1))
            psg = qs.enter_context(tc.tile_pool(name=f"psg{b}", bufs=8,
                                                space="PSUM"))
            # (w1-quarter0, x) DMA pairs first - the GEMM1 ramp consumes them
            # in arrival order; later quarters stream behind their passes.
            xs_pool = qs.enter_context(tc.tile_pool(name=f"xs{b}", bufs=3))
            x_sb = [x_pool.tile([128, HW], F32R, name=f"xsb{b}_{k}",
                                tag=f"xsb{k}") for k in range(8)]
            q0 = [w1_pool.tile([128, 512], F32R, name=f"w1q{b}_0_{k}",
                               tag=f"qbuf0_{k}") for k in range(8)]
            xl_d, xh_d = x_d
            for k in range(8):
                nc.sync.dma_start(q0[k][:], w1_d[0, 128 * k:128 * k + 128, :])
                stage = xs_pool.tile([128, HW], BF16, name=f"xst{b}_{k}",
                                     tag="xstage")
                nc.sync.dma_start(stage[:, 0:512],
                                  xl_d[b, 128 * k:128 * k + 128, :])
                nc.sync.dma_start(stage[:, 512:1024],
                                  xh_d[b, 128 * k:128 * k + 128, :])
                nc.scalar.activation(x_sb[k][:], stage[:], AF.Identity)
            w1quads = [q0] + [load_w1_quarter(n) for n in range(1, 4)]
            if b == 0:
                for k in range(8):
                    nc.sync.dma_start(w2_sb[k][:], w2_d[128 * k:128 * k + 128, :])
                for k in range(8):
                    nc.sync.dma_start(wp_sb[k][:], wp_d[128 * k:128 * k + 128, :])

            # GEMM1 (q,k): quarter pass 0 k-outer (consumes DMA pairs in
            # arrival order); passes 1-3 k-inner (back-to-back accumulation
            # is ~75ns/matmul faster than bank-interleaved k-outer)
            pss = [psg.tile([128, 512], F32, name=f"psg1_{b}_0_{m}",
                            tag="psg") for m in range(8)]
            for k in range(8):
                for m in range(8):
                    nc.tensor.matmul(
                        pss[m][:],
                        x_sb[k][:, 128 * m:128 * m + 128],
                        w1quads[0][k][:],
                        start=(k == 0), stop=(k == 7))
            for m in range(8):
                nc.vector.tensor_add(qkT[m][:, 0:512], pss[m][:],
                                     b1_bc[:, 0:512])
            for n in range(1, 4):
                w1q = w1quads[n]
                for m in range(8):
                    ps = psg.tile([128, 512], F32, name=f"psg1_{b}_{n}_{m}",
                                  tag="psg")
                    for k in range(8):
                        nc.tensor.matmul(
                            ps[:],
                            x_sb[k][:, 128 * m:128 * m + 128],
                            w1q[k][:],
                            start=(k == 0), stop=(k == 7))
                    nc.vector.tensor_add(qkT[m][:, 512 * n:512 * n + 512],
                                         ps[:], b1_bc[:, 512 * n:512 * n + 512])

            # GEMM2 (v): k-inner
            for m in range(8):
                for n in range(2):
                    ps = psg.tile([128, 512], F32, name=f"psg2_{b}_{m}_{n}",
                                  tag="psg")
                    for k in range(8):
                        nc.tensor.matmul(
                            ps[:],
                            w2_sb[k][:, 128 * m:128 * m + 128],
                            x_sb[k][:, 512 * n:512 * n + 512],
                            start=(k == 0), stop=(k == 7))
                    nc.scalar.activation(v_sb[m][:, 512 * n:512 * n + 512],
                                         ps[:], AF.Identity, bias=b2_sb[:, m:m + 1])

        # ---------------- attention ----------------
        ao_pool = bs.enter_context(tc.tile_pool(name=f"ao{b}", bufs=1))
        ao_sb = [ao_pool.tile([128, 1024], F32R, name=f"aosb{b}_{m}", tag=f"ao{m}")
                 for m in range(8)]

        att = bs.enter_context(tile.ExitStack())
        e_pool = att.enter_context(tc.tile_pool(name=f"e{b}", bufs=2))
        r_pool = att.enter_context(tc.tile_pool(name=f"r{b}", bufs=4))
        ps_st = att.enter_context(tc.tile_pool(name=f"pst{b}", bufs=4, space="PSUM"))
        ps_pv = att.enter_context(tc.tile_pool(name=f"ppv{b}", bufs=2, space="PSUM"))

        def attn_st(h):
            g, half = h // 2, h % 2
            base = 4 * half
            es = []
            for kt in range(2):
                ps = ps_st.tile([128, 256], F32, name=f"ps_st{b}_{h}_{kt}",
                                tag="ps_st")
                for d in range(4):
                    nc.tensor.matmul(
                        ps[:],
                        qkT[base + d][:, (4 + g) * 256 + 128 * kt:
                                      (4 + g) * 256 + 128 * kt + 128],
                        qkT[base + d][:, g * 256:g * 256 + 256],
                        start=(d == 0), stop=(d == 3))
                e = e_pool.tile([128, 256], BF16, name=f"E{b}_{h}_{kt}",
                                tag=f"E{kt}")
                nc.scalar.activation(e[:], ps[:], AF.Exp)
                es.append(e)
            return es

        def attn_pv(h, es):
            g, half = h // 2, h % 2
            for qt in range(2):
                psO = ps_pv.tile([128, 512], F32, name=f"psO{b}_{h}_{qt}", tag="psO")
                psL = ps_pv.tile([128, 8], F32, name=f"psL{b}_{h}_{qt}", tag="psL")
                for kt in range(2):
                    nc.tensor.matmul(
                        psO[:], es[kt][:, 128 * qt:128 * qt + 128],
                        v_sb[2 * g + kt][:, 512 * half:512 * half + 512],
                        start=(kt == 0), stop=(kt == 1))
                    nc.tensor.matmul(
                        psL[:], es[kt][:, 128 * qt:128 * qt + 128],
                        ones_col[:, 0:8],
                        start=(kt == 0), stop=(kt == 1))
                r = r_pool.tile([128, 1], F32, name=f"r{b}_{h}_{qt}", tag="r")
                nc.vector.reciprocal(r[:], psL[:, 0:1])
                dst = ao_sb[2 * g + qt]
                nc.vector.tensor_scalar_mul(
                    dst[:, 512 * half:512 * half + 512], psO[:], r[:])

        es_next = attn_st(0)
        for h in range(NH):
            es_cur = es_next
            es_next = attn_st(h + 1) if h + 1 < NH else None
            attn_pv(h, es_cur)
        att.close()

        # ---------------- proj GEMM: k-inner ----------------
        with tile.ExitStack() as pjs:
            psp = pjs.enter_context(tc.tile_pool(name=f"psp{b}", bufs=4,
                                                 space="PSUM"))
            for m in range(8):
                for n in range(2):
                    ps = psp.tile([128, 512], F32, name=f"psp{b}_{m}_{n}",
                                  tag="psp")
                    for k in range(8):
                        nc.tensor.matmul(
                            ps[:],
                            wp_sb[k][:, 128 * m:128 * m + 128],
                            ao_sb[k][:, 512 * n:512 * n + 512],
                            start=(k == 0), stop=(k == 7))
                    y_sb = y_pool.tile([128, 512], BF16, name=f"ysb{b}_{m}_{n}",
                                       tag="ysb")
                    if (2 * m + n) % 2 == 0:
                        nc.scalar.activation(y_sb[:], ps[:], AF.Identity,
                                             bias=bp_sb[:, m:m + 1])
                    else:
                        nc.vector.tensor_scalar_add(y_sb[:], ps[:],
                                                    bp_sb[:, m:m + 1])
                    nc.sync.dma_start(
                        y_d[b, 128 * m:128 * m + 128, 512 * n:512 * n + 512],
                        y_sb[:])


def _prepare_host_inputs(w_qkv, b_qkv, w_proj):
    """Permute weights so device layouts need no transposes. See layout notes."""
    C = CIN
    scale = np.float32((C // NH) ** -0.5)
    g_i, p_i = np.meshgrid(np.arange(4), np.arange(256), indexing="ij")
    # GEMM1 columns: (t, g, p) -> channel 12p + 4t + g
    t_i, g2_i, p2_i = np.meshgrid(np.arange(2), np.arange(4), np.arange(256),
                                  indexing="ij")
    src1 = (12 * p2_i + 4 * t_i + g2_i).reshape(-1)
    w1 = w_qkv[src1, :].astype(np.float32).copy()
    b1 = b_qkv[src1].astype(np.float32).copy()
    w1[:1024] *= scale
    b1[:1024] *= scale
    w1t = np.ascontiguousarray(w1.T)                       # [1024, 2048]
    # GEMM2 rows: r = g*256 + p -> channel 12p + 8 + g
    src2 = (12 * p_i + 8 + g_i).reshape(-1)
    w2t = np.ascontiguousarray(w_qkv[src2, :].T.astype(np.float32))   # [1024, 1024]
    b2 = b_qkv[src2].astype(np.float32).copy()
    # proj contraction: c' = g*256 + p -> orig col 4p + g
    srcp = (4 * p_i + g_i).reshape(-1)
    wpt = np.ascontiguousarray(w_proj[:, srcp].T.astype(np.float32))  # [1024, 1024]
    return w1t, b1, w2t, b2, wpt


def kernel(x, w_qkv, b_qkv, w_proj, b_proj):
    if "nc" not in _CACHE:
        _CACHE["nc"] = _build_program()
    nc = _CACHE["nc"]

    x = np.asarray(x, dtype=np.float32)
    B = x.shape[0]
    xf = x.reshape(B, CIN, HW)
    xl = np.ascontiguousarray(xf[:, :, 0:512]).astype(BF16_NP)
    xh = np.ascontiguousarray(xf[:, :, 512:1024]).astype(BF16_NP)
    w1t, b1, w2t, b2, wpt = _prepare_host_inputs(
        np.asarray(w_qkv, np.float32), np.asarray(b_qkv, np.float32),
        np.asarray(w_proj, np.float32))
    # w1 as four contiguous quarter tensors [4, 1024, 512]
    w1q = np.ascontiguousarray(w1t.reshape(CIN, 4, 512).transpose(1, 0, 2))
    b1r = np.ascontiguousarray(np.tile(b1.reshape(1, 2048), (128, 1)))
    bp = np.asarray(b_proj, np.float32)
    ones_c = np.ones((128, 8), BF16_NP)

    in_maps = []
    for c in range(N_CORES):
        in_maps.append({
            "xl": xl[c * B_PER_CORE:(c + 1) * B_PER_CORE],
            "xh": xh[c * B_PER_CORE:(c + 1) * B_PER_CORE],
            "w1q": w1q, "w2t": w2t, "wpt": wpt,
            "b1r": b1r, "b2": b2, "bp": bp,
            "ones_c": ones_c,
        })
    res = bass_utils.run_bass_kernel_spmd(nc, in_maps, core_ids=list(range(N_CORES)))
    _CACHE["last_results"] = res
    y = np.concatenate([np.asarray(res.results[c]["y"], dtype=np.float32)
                        for c in range(N_CORES)], axis=0)
    return np.ascontiguousarray(y.reshape(B, CIN, 32, 32))
